# revision 1
# baseline (speedup 1.0000x reference)
"""Causal self-attention (B=4, S=2048, D=768, H=12) on 8 TRN2 NeuronCores.

Sharding: batch (4) x head-group (2) = 8 cores.  Each core computes, for its
batch b and 6 heads:
  - x^T via PE transposes (contraction over D needs D on partitions)
  - QK^T projection directly in transposed layout (head-dim on partitions),
    Q pre-scaled by 1/sqrt(dh) via host-side weight scaling
  - V projection in natural layout, with a ones column appended per head
    (so the AV matmul also produces softmax denominators for free)
  - flash-style causal attention with scores kept transposed
    (S^T = K Q^T): softmax needs no max-subtraction (scores are O(1) here),
    exp on ACT, causal mask as a 0/1 multiply on diagonal blocks only
  - AV^T accumulated in PSUM over key chunks -> O^T [dh, q] per head,
    normalized by PE-broadcast reciprocal of the fused sums row
  - partial output projection (its 384 rows of W_out)
Host: sums the two partial outputs per batch and adds the constant
b_v @ W_out + b_out (V-bias commutes through softmax-normalized attention).

All matmuls run in float32r (full-rate fp32 on the PE, ~1.2e-4 rounding).
"""

import numpy as np

import concourse.bass as bass
import concourse.tile as tile
import concourse.mybir as mybir
from concourse import bacc
from concourse._compat import with_exitstack  # noqa: F401  (parity with repo kernels)

F32 = mybir.dt.float32
F32R = mybir.dt.float32r

B, S, D = 4, 2048, 768
H, DH = 12, 64
G = 2                 # head groups (tensor-parallel dimension)
HPG = H // G          # heads per group = 6
NPAIR = HPG // 2      # head pairs per group = 3
N_CORES = 8
ST = 128              # S-tile for projections / output rows
QT = 512              # q-tile for attention
KC = 128              # key chunk
N_ST = S // ST        # 16
N_QT = S // QT        # 4
DC = D // 128         # 6 contraction chunks over D


def declare_io(nc):
    """DRAM tensors; names must match in_maps keys."""
    io = {}
    io["x"] = nc.dram_tensor("x", [S, D], F32R, kind="ExternalInput")
    io["wqk"] = nc.dram_tensor("wqk", [D, 768], F32R, kind="ExternalInput")
    io["bqk2"] = nc.dram_tensor("bqk2", [128, 6], F32, kind="ExternalInput")
    io["wv"] = nc.dram_tensor("wv", [D, 384], F32R, kind="ExternalInput")
    io["wo"] = nc.dram_tensor("wo", [384, 768], F32R, kind="ExternalInput")
    io["masks"] = nc.dram_tensor("masks", [2, KC, QT], F32R, kind="ExternalInput")
    io["ident"] = nc.dram_tensor("ident", [128, 128], F32R, kind="ExternalInput")
    io["sel"] = nc.dram_tensor("sel", [128, 128], F32R, kind="ExternalInput")
    io["ones2"] = nc.dram_tensor("ones2", [128, HPG], F32R, kind="ExternalInput")
    io["out"] = nc.dram_tensor("out", [S, D], F32, kind="ExternalOutput")
    return io


def build_body(nc, tc, pools, io, phases=(1, 2, 3, 4, 5)):
    """Emit one full forward pass (per-core program)."""
    (consts, w768, wsmall, slab, qkT_p, vsb_p, xload, psA, psB, scp, pT_p,
     rcp_p, atmp_p, outsb_p) = pools

    # ---- constants / weights into SBUF ----
    ident_t = consts.tile([128, 128], F32R, tag="ident")
    nc.sync.dma_start(out=ident_t, in_=io["ident"][:])
    sel_t = consts.tile([128, 128], F32R, tag="sel")
    nc.sync.dma_start(out=sel_t, in_=io["sel"][:])
    bqk2_t = consts.tile([128, 6], F32, tag="bqk2")
    nc.sync.dma_start(out=bqk2_t, in_=io["bqk2"][:])
    masks_t = []
    for r in range(2):
        m = consts.tile([KC, QT], F32R, tag=f"mask{r}")
        nc.sync.dma_start(out=m, in_=io["masks"][r])
        masks_t.append(m)

    wqk_t = []
    for c in range(DC):
        w = w768.tile([128, 768], F32R, tag="w768")
        nc.sync.dma_start(out=w, in_=io["wqk"][c * 128:(c + 1) * 128, :])
        wqk_t.append(w)
    wv_t = []
    for c in range(DC):
        w = wsmall.tile([128, 384], F32R, tag="wv")
        nc.sync.dma_start(out=w, in_=io["wv"][c * 128:(c + 1) * 128, :])
        wv_t.append(w)

    # ---- x^T (6 tiles [128, S]) via PE transposes, 4 S-tiles per copy ----
    xT = [slab.tile([128, S], F32R, tag="slab", name=f"xT{c}") for c in range(DC)]
    for s4 in range(N_ST // 4):
        xts = []
        for si in range(4):
            s = s4 * 4 + si
            xt = xload.tile([128, D], F32R, tag="xload")
            nc.sync.dma_start(out=xt, in_=io["x"][s * ST:(s + 1) * ST, :])
            xts.append(xt)
        for c in range(DC):
            tp = psA.tile([128, 512], F32R, tag="ps1")
            for si in range(4):
                nc.tensor.transpose(tp[:, si * 128:(si + 1) * 128],
                                    xts[si][:, c * 128:(c + 1) * 128], ident_t[:])
            nc.vector.tensor_copy(xT[c][:, s4 * 512:(s4 + 1) * 512], tp)

    if 2 not in phases:
        return
    # ---- QK^T projection: qkT[j] j even = Q-pair j//2, j odd = K-pair ----
    qkT = [qkT_p.tile([128, S], F32R, tag="qkT", name=f"qkT{j}") for j in range(6)]
    for j in range(6):
        for t in range(N_QT):
            pp = psA.tile([128, QT], F32, tag="ps1")
            for c in range(DC):
                nc.tensor.matmul(pp, wqk_t[c][:, j * 128:(j + 1) * 128],
                                 xT[c][:, t * QT:(t + 1) * QT],
                                 start=(c == 0), stop=(c == DC - 1))
            nc.vector.tensor_scalar_add(qkT[j][:, t * QT:(t + 1) * QT], pp,
                                        bqk2_t[:, j:j + 1])

    if 3 not in phases:
        return
    # ---- V projection into [V_h | ones] blocks of 65 cols ----
    vsb = []
    for s in range(N_ST):
        vp = psA.tile([128, 384], F32, tag="ps1")
        for c in range(DC):
            nc.tensor.matmul(vp, xT[c][:, s * ST:(s + 1) * ST], wv_t[c][:],
                             start=(c == 0), stop=(c == DC - 1))
        vv = vsb_p.tile([128, HPG, 65], F32R, tag="vsb")
        nc.vector.tensor_copy(vv[:, :, 0:64],
                              vp[:].rearrange("p (h d) -> p h d", h=HPG))
        nc.sync.dma_start(out=vv[:, :, 64:65],
                          in_=io["ones2"][:].rearrange("p (h o) -> p h o", o=1))
        vsb.append(vv)

    if 4 not in phases:
        return
    # ---- attention ----
    apair = [slab.tile([128, S], F32R, tag="slab", name=f"apair{p}") for p in range(NPAIR)]
    for p in range(NPAIR):
        qp = qkT[2 * p]
        kp = qkT[2 * p + 1]
        for t in range(N_QT):
            n_kc = 4 * t + 4
            av_e = psB.tile([65, QT], F32, tag="ps2")
            av_o = psB.tile([65, QT], F32, tag="ps2")
            avs = (av_e, av_o)

            def chunk_geom(kc):
                # causal slicing: diagonal chunk r only touches q-window
                # [off, 512); r==3 widened to 256 so fp32r stays full-rate.
                r = kc - 4 * t
                if r < 0:
                    return 0, QT, None
                if r < 3:
                    return 128 * r, QT - 128 * r, masks_t[0][:, 0:128]
                return 256, 256, masks_t[1][:, 0:256]

            for kc in range(n_kc):
                off, w, msk = chunk_geom(kc)
                # both heads' scores in one 2-bank PSUM tile so exp and the
                # causal-mask multiply run once per chunk pair (ACT per-op
                # overhead is ~300ns; halving the op count matters)
                sc2 = scp.tile([KC, 2, QT], F32, tag="sc2")
                pt2 = pT_p.tile([KC, 2, QT], F32R, tag="pT")
                for j in (0, 1):
                    nc.tensor.matmul(
                        sc2[:, j, 0:w],
                        kp[j * 64:(j + 1) * 64, kc * KC:(kc + 1) * KC],
                        qp[j * 64:(j + 1) * 64, t * QT + off:(t + 1) * QT],
                        start=True, stop=True, tile_position=(j * 64, 0))
                nc.scalar.activation(pt2[:, :, 0:w], sc2[:, :, 0:w],
                                     mybir.ActivationFunctionType.Exp)
                if msk is not None:
                    mw = msk.shape[1]
                    msk2 = bass.AP(tensor=msk.tensor, offset=msk.offset,
                                   ap=[list(msk.ap[0]), [0, 2], list(msk.ap[1])])
                    nc.vector.tensor_mul(pt2[:, :, 0:mw], pt2[:, :, 0:mw], msk2)
                for j in (0, 1):
                    nc.tensor.matmul(
                        avs[j][:, off:off + w], vsb[kc][:, 2 * p + j, :],
                        pt2[:, j, 0:w],
                        start=(kc == 0), stop=(kc == n_kc - 1))
            rc_e = rcp_p.tile([65, QT], F32R, tag="rcp")
            nc.vector.reciprocal(rc_e[64:65, :], av_e[64:65, :])
            rc_o = rcp_p.tile([65, QT], F32R, tag="rcp")
            nc.vector.reciprocal(rc_o[64:65, :], av_o[64:65, :])
            bc_e = psA.tile([64, QT], F32, tag="ps1")
            nc.tensor.matmul(bc_e, sel_t[64:65, 0:64], rc_e[64:65, :],
                             start=True, stop=True)
            bc_e_sb = rcp_p.tile([64, QT], F32, tag="bcsb")
            nc.vector.tensor_copy(bc_e_sb, bc_e)
            bc_o = psA.tile([64, QT], F32, tag="ps1")
            nc.tensor.matmul(bc_o, sel_t[64:65, 0:64], rc_o[64:65, :],
                             start=True, stop=True)
            bc_o_sb = rcp_p.tile([64, QT], F32, tag="bcsb")
            nc.vector.tensor_copy(bc_o_sb, bc_o)
            nc.vector.tensor_mul(apair[p][0:64, t * QT:(t + 1) * QT],
                                 av_e[0:64, :], bc_e_sb[:])
            at = atmp_p.tile([64, QT], F32R, tag="atmp")
            nc.vector.tensor_mul(at, av_o[0:64, :], bc_o_sb[:])
            nc.sync.dma_start(out=apair[p][64:128, t * QT:(t + 1) * QT], in_=at)

    if 5 not in phases:
        return
    # ---- output projection (partial: this group's 384 rows of W_out) ----
    wo_t = []
    for p in range(NPAIR):
        w = w768.tile([128, 768], F32R, tag="w768")
        nc.sync.dma_start(out=w, in_=io["wo"][p * 128:(p + 1) * 128, :])
        wo_t.append(w)
    for s in range(N_ST):
        o1 = psA.tile([128, 512], F32, tag="ps1")
        o2 = psA.tile([128, 256], F32, tag="ps1")
        for p in range(NPAIR):
            nc.tensor.matmul(o1, apair[p][:, s * ST:(s + 1) * ST],
                             wo_t[p][:, 0:512],
                             start=(p == 0), stop=(p == NPAIR - 1))
        for p in range(NPAIR):
            nc.tensor.matmul(o2, apair[p][:, s * ST:(s + 1) * ST],
                             wo_t[p][:, 512:768],
                             start=(p == 0), stop=(p == NPAIR - 1))
        osb = outsb_p.tile([128, D], F32, tag="outsb")
        nc.vector.tensor_copy(osb[:, 0:512], o1)
        nc.vector.tensor_copy(osb[:, 512:768], o2)
        nc.sync.dma_start(out=io["out"][s * ST:(s + 1) * ST, :], in_=osb)


def make_pools(tc, ctx):
    consts = ctx.enter_context(tc.tile_pool(name="consts", bufs=1))
    w768 = ctx.enter_context(tc.tile_pool(name="w768", bufs=6))
    wsmall = ctx.enter_context(tc.tile_pool(name="wsmall", bufs=6))
    slab = ctx.enter_context(tc.tile_pool(name="slab", bufs=6))
    qkT_p = ctx.enter_context(tc.tile_pool(name="qkT", bufs=6))
    vsb_p = ctx.enter_context(tc.tile_pool(name="vsb", bufs=16))
    xload = ctx.enter_context(tc.tile_pool(name="xload", bufs=5))
    psA = ctx.enter_context(tc.tile_pool(name="psA", bufs=2, space="PSUM"))
    psB = ctx.enter_context(tc.tile_pool(name="psB", bufs=2, space="PSUM"))
    scp = ctx.enter_context(tc.tile_pool(name="scp", bufs=2, space="PSUM"))
    pT_p = ctx.enter_context(tc.tile_pool(name="pT", bufs=3))
    rcp_p = ctx.enter_context(tc.tile_pool(name="rcp", bufs=2))
    atmp_p = ctx.enter_context(tc.tile_pool(name="atmp", bufs=2))
    outsb_p = ctx.enter_context(tc.tile_pool(name="outsb", bufs=2))
    return (consts, w768, wsmall, slab, qkT_p, vsb_p, xload, psA, psB, scp,
            pT_p, rcp_p, atmp_p, outsb_p)


def build_nc(n_iters=None, phases=(1, 2, 3, 4, 5)):
    """Build the per-core program. n_iters wraps the body in a HW loop
    (timing harness only; the graded path uses n_iters=None)."""
    from contextlib import ExitStack

    nc = bacc.Bacc(trn_type="TRN2", debug=False)
    nc._allow_low_precision_reason = "float32r matmuls keep fp32 width"
    io = declare_io(nc)
    with tile.TileContext(nc) as tc:
        with ExitStack() as ctx:
            pools = make_pools(tc, ctx)
            if n_iters is None:
                build_body(nc, tc, pools, io, phases)
            else:
                with tc.For_i(0, n_iters, 1):
                    build_body(nc, tc, pools, io, phases)
    nc.compile()
    return nc, io


def host_inputs(x, W_qkv, b_qkv, W_out, b_out):
    """Per-core in_maps + the host-side unshard constant."""
    x = np.asarray(x, dtype=np.float32)
    W_qkv = np.asarray(W_qkv, dtype=np.float32)
    b_qkv = np.asarray(b_qkv, dtype=np.float32)
    W_out = np.asarray(W_out, dtype=np.float32)
    b_out = np.asarray(b_out, dtype=np.float32)

    Wq, Wk, Wv = W_qkv[:, 0:D], W_qkv[:, D:2 * D], W_qkv[:, 2 * D:3 * D]
    bq, bk, bv = b_qkv[0:D], b_qkv[D:2 * D], b_qkv[2 * D:3 * D]
    scale = 1.0 / np.sqrt(DH)

    # shared constants
    masks = np.zeros((2, KC, QT), np.float32)
    for r in range(2):
        kk = np.arange(KC)[:, None]
        qq = np.arange(QT)[None, :]
        masks[r] = (qq >= kk + KC * r).astype(np.float32)
    ident = np.eye(128, dtype=np.float32)
    sel = np.zeros((128, 128), np.float32)
    sel[64, 0:64] = 1.0
    ones2 = np.ones((128, HPG), np.float32)

    per_group = []
    for g in range(G):
        cols = []
        bcols = []
        for p in range(NPAIR):
            h0 = g * HPG + 2 * p
            h1 = h0 + 1
            cols.append(Wq[:, h0 * DH:(h0 + 2) * DH] * scale)   # q-pair
            cols.append(Wk[:, h0 * DH:(h0 + 2) * DH])           # k-pair
            bcols.append(bq[h0 * DH:(h0 + 2) * DH] * scale)
            bcols.append(bk[h0 * DH:(h0 + 2) * DH])
        wqk_g = np.concatenate(cols, axis=1)                    # [768, 768]
        bqk_g = np.stack(bcols, axis=1)                         # [128, 6]
        wv_g = Wv[:, g * HPG * DH:(g + 1) * HPG * DH]           # [768, 384]
        wo_g = W_out[g * HPG * DH:(g + 1) * HPG * DH, :]        # [384, 768]
        per_group.append((wqk_g, bqk_g, wv_g, wo_g))

    in_maps = []
    for core in range(N_CORES):
        b, g = core // G, core % G
        wqk_g, bqk_g, wv_g, wo_g = per_group[g]
        in_maps.append(dict(
            x=np.ascontiguousarray(x[b]),
            wqk=np.ascontiguousarray(wqk_g),
            bqk2=np.ascontiguousarray(bqk_g),
            wv=np.ascontiguousarray(wv_g),
            wo=np.ascontiguousarray(wo_g),
            masks=masks, ident=ident, sel=sel,
            ones2=ones2,
        ))
    cvec = (bv @ W_out + b_out).astype(np.float32)              # [768]
    return in_maps, cvec


_CACHE = {}


def kernel(x, W_qkv, b_qkv, W_out, b_out):
    from concourse.bass_utils import run_bass_kernel_spmd

    if "nc" not in _CACHE:
        _CACHE["nc"], _ = build_nc()
    nc = _CACHE["nc"]
    in_maps, cvec = host_inputs(x, W_qkv, b_qkv, W_out, b_out)
    res = run_bass_kernel_spmd(nc, in_maps, list(range(N_CORES)))
    out = np.empty((B, S, D), np.float32)
    for b in range(B):
        out[b] = res.results[2 * b]["out"] + res.results[2 * b + 1]["out"] + cvec
    return out



# revision 5
# speedup vs baseline: 1.1719x; 1.1719x over previous
"""Causal self-attention (B=4, S=2048, D=768, H=12) on 8 TRN2 NeuronCores.

Sharding: batch (4) x head-group (2) = 8 cores.  Each core computes its
batch's 6 heads and a partial output projection; host sums the two group
partials plus the constant (b_v @ W_out + b_out).

v2 layout (vs baseline): everything bf16 on-device except PSUM accumulation
(f32), biases and the final output tile.  x arrives pre-transposed from the
host (xt = x.T in bf16), eliminating the PE transpose phase.  The attention
inner loop is software-pipelined: the AV matmul for chunk k is emitted after
the score matmul for chunk k+1, so the PE never sits behind the ACT exp on
the in-order queue.  QK/V/output projections are emitted as "filler" units
interleaved between attention chunks so the PE's projection work overlaps
the ACT-bound softmax stream.  q-tile QT=256 keeps each score tile in one
PSUM bank: 3 score bufs + 2 av bufs + 3 misc bufs = 8 banks exactly.
"""

import numpy as np

import concourse.bass as bass
import concourse.tile as tile
import concourse.mybir as mybir
from concourse import bacc
from concourse._compat import with_exitstack  # noqa: F401

F32 = mybir.dt.float32
BF16 = mybir.dt.bfloat16

B, S, D = 4, 2048, 768
H, DH = 12, 64
G = 2                  # head groups (tensor-parallel dimension)
HPG = H // G           # heads per group = 6
NPAIR = HPG // 2       # head pairs per group = 3
N_CORES = 8
QT = 256               # q-tile for attention
KC = 128               # key chunk
N_QT = S // QT         # 8
DC = D // 128          # 6 contraction chunks over D
WVC = HPG * (DH + 1)   # 390: V proj output cols ([V_h | 0] x 6 heads)

ACT_CYC = 0.833
PE_CYC = 0.4167


def declare_io(nc):
    io = {}
    io["xt"] = nc.dram_tensor("xt", [D, S], BF16, kind="ExternalInput")
    io["wqk"] = nc.dram_tensor("wqk", [D, 768], BF16, kind="ExternalInput")
    io["bqk2"] = nc.dram_tensor("bqk2", [128, 6], F32, kind="ExternalInput")
    io["wv"] = nc.dram_tensor("wv", [D, WVC], BF16, kind="ExternalInput")
    io["wo"] = nc.dram_tensor("wo", [384, 768], BF16, kind="ExternalInput")
    io["mask"] = nc.dram_tensor("mask", [KC, KC], BF16, kind="ExternalInput")
    io["sel"] = nc.dram_tensor("sel", [65, 64], BF16, kind="ExternalInput")
    io["onesc"] = nc.dram_tensor("onesc", [128, HPG], BF16, kind="ExternalInput")
    io["out"] = nc.dram_tensor("out", [S, D], F32, kind="ExternalOutput")
    return io


def build_body(nc, tc, pools, io):
    (consts, wqk_p, wv_p, wo_p, xt_p, qkT_p, vsb_p, pT_p, apair_p, rc_p,
     bcsb_p, at_p, outsb_p, ps_sc, ps_av, ps_ms) = pools

    # ---- constants / weights into SBUF ----
    mask_t = consts.tile([KC, KC], BF16, tag="mask")
    nc.sync.dma_start(out=mask_t, in_=io["mask"][:])
    sel_t = consts.tile([65, 64], BF16, tag="sel")
    nc.sync.dma_start(out=sel_t, in_=io["sel"][:])
    bqk2_t = consts.tile([128, 6], F32, tag="bqk2")
    nc.sync.dma_start(out=bqk2_t, in_=io["bqk2"][:])
    onesc_t = consts.tile([128, HPG], BF16, tag="onesc")
    nc.sync.dma_start(out=onesc_t, in_=io["onesc"][:])

    wqk_t, xt_t = [], []
    for c in range(DC):
        w = wqk_p.tile([128, 768], BF16, tag="wqk", name=f"wqk{c}")
        nc.sync.dma_start(out=w, in_=io["wqk"][c * 128:(c + 1) * 128, :])
        wqk_t.append(w)
        x = xt_p.tile([128, S], BF16, tag="xt", name=f"xt{c}")
        nc.sync.dma_start(out=x, in_=io["xt"][c * 128:(c + 1) * 128, :])
        xt_t.append(x)
    wv_t = []
    for c in range(DC):
        w = wv_p.tile([128, WVC], BF16, tag="wv", name=f"wv{c}")
        nc.sync.dma_start(out=w, in_=io["wv"][c * 128:(c + 1) * 128, :])
        wv_t.append(w)
    wo_t = []
    for p in range(NPAIR):
        w = wo_p.tile([128, 768], BF16, tag="wo", name=f"wo{p}")
        nc.sync.dma_start(out=w, in_=io["wo"][p * 128:(p + 1) * 128, :])
        wo_t.append(w)

    qkT = [qkT_p.tile([128, S], BF16, tag="qkT", name=f"qkT{j}")
           for j in range(6)]
    vsb = [vsb_p.tile([128, HPG, DH + 1], BF16, tag="vsb", name=f"vsb{s}")
           for s in range(S // 128)]
    apair = [apair_p.tile([128, S], BF16, tag="apair", name=f"apair{p}")
             for p in range(NPAIR)]

    # ---- emission units ----
    def emit_qk_unit(j, u):
        # qkT[j][:, 512u:512(u+1)] = (x W)^T for q/k pair column j
        pp = ps_ms.tile([128, 512], F32, tag="ms", name=f"pp{j}_{u}")
        for c in range(DC):
            nc.tensor.matmul(pp, wqk_t[c][:, j * 128:(j + 1) * 128],
                             xt_t[c][:, u * 512:(u + 1) * 512],
                             start=(c == 0), stop=(c == DC - 1))
        nc.vector.tensor_scalar_add(qkT[j][:, u * 512:(u + 1) * 512], pp,
                                    bqk2_t[:, j:j + 1])

    def emit_v_unit(s):
        vp = ps_ms.tile([128, WVC], F32, tag="ms", name=f"vp{s}")
        for c in range(DC):
            nc.tensor.matmul(vp, xt_t[c][:, s * 128:(s + 1) * 128], wv_t[c][:],
                             start=(c == 0), stop=(c == DC - 1))
        vv = vsb[s]
        nc.vector.tensor_copy(
            vv[:].rearrange("p h d -> p (h d)"), vp)
        nc.vector.tensor_copy(vv[:, :, DH:DH + 1],
                              onesc_t[:].rearrange("p (h o) -> p h o", o=1))

    def emit_out_unit(s):
        o1 = ps_ms.tile([128, 512], F32, tag="ms", name=f"o1_{s}")
        o2 = ps_ms.tile([128, 256], F32, tag="ms", name=f"o2_{s}")
        for p in range(NPAIR):
            nc.tensor.matmul(o1, apair[p][:, s * 128:(s + 1) * 128],
                             wo_t[p][:, 0:512],
                             start=(p == 0), stop=(p == NPAIR - 1))
        for p in range(NPAIR):
            nc.tensor.matmul(o2, apair[p][:, s * 128:(s + 1) * 128],
                             wo_t[p][:, 512:768],
                             start=(p == 0), stop=(p == NPAIR - 1))
        osb = outsb_p.tile([128, D], F32, tag="outsb", name=f"osb{s}")
        nc.vector.tensor_copy(osb[:, 0:512], o1)
        nc.vector.tensor_copy(osb[:, 512:768], o2)
        nc.sync.dma_start(out=io["out"][s * 128:(s + 1) * 128, :], in_=osb)

    # ---- filler scheduling ----
    # (deadline, pe_ns, emit_fn); deadline = (pair, t) before which the unit
    # must run.  Emission order = deadline order; pacing spreads them early.
    fillers = []

    def add_qk(j, u):
        pair = j // 2
        dl = (pair, min(2 * u, N_QT - 1))
        fillers.append([dl, 6 * 512 * PE_CYC, lambda: emit_qk_unit(j, u)])

    def add_v(s):
        dl = (0, max(0, (s - 1) // 2))
        fillers.append([dl, 6 * WVC * PE_CYC, lambda: emit_v_unit(s)])

    for s in range(2, 16):
        add_v(s)
    for u in range(1, 4):
        add_qk(0, u)
        add_qk(1, u)
    for pair in (1, 2):
        for u in range(4):
            add_qk(2 * pair, u)
            add_qk(2 * pair + 1, u)
    fillers.sort(key=lambda f: f[0])

    total_act = 0.0
    for t in range(N_QT):
        for kc in range(2 * t + 2):
            w = QT if kc < 2 * t else QT - KC * (kc - 2 * t)
            total_act += 2 * w * ACT_CYC + 143.0
    total_act *= NPAIR

    state = dict(act_done=0.0, fill_done=0.0, fill_total=sum(f[1] for f in fillers))

    def maybe_fill(force_deadline=None):
        while fillers:
            dl, pe_ns, fn = fillers[0]
            forced = force_deadline is not None and dl <= force_deadline
            paced = (state["fill_done"] < state["fill_total"]
                     * (state["act_done"] / total_act) + 2000.0)
            if not (forced or paced):
                break
            fillers.pop(0)
            fn()
            state["fill_done"] += pe_ns

    # ---- lead-in ----
    emit_qk_unit(0, 0)
    emit_qk_unit(1, 0)
    emit_v_unit(0)
    emit_v_unit(1)

    # ---- attention driver ----
    pending_norm = [None]

    def emit_norm_a(av2):
        # reciprocal of the fused sums row (partition 64), both heads
        rc = rc_p.tile([65, 2, QT], BF16, tag="rc")
        nc.vector.reciprocal(rc[64:65, :, :], av2[64:65, :, :])
        return rc

    def emit_norm_b(p, t, av2, rc):
        # broadcast recip down 64 partitions via PE, then scale + store
        bc2 = ps_ms.tile([64, 2, QT], F32, tag="ms", name=f"bc{p}_{t}")
        for j in range(2):
            nc.tensor.matmul(bc2[:, j, :], sel_t[64:65, 0:64],
                             rc[64:65, j, :], start=True, stop=True)
        bcs = bcsb_p.tile([64, 2, QT], BF16, tag="bcsb", name=f"bcs{p}_{t}")
        nc.vector.tensor_copy(bcs, bc2)
        nc.vector.tensor_mul(apair[p][0:64, t * QT:(t + 1) * QT],
                             av2[0:64, 0, :], bcs[:, 0, :])
        at = at_p.tile([64, QT], BF16, tag="at", name=f"at{p}_{t}")
        nc.vector.tensor_mul(at, av2[0:64, 1, :], bcs[:, 1, :])
        nc.sync.dma_start(out=apair[p][64:128, t * QT:(t + 1) * QT], in_=at)
        if p == 2 and t % 2 == 1:
            u = t // 2
            for s in range(4 * u, 4 * u + 4):
                fillers.append([(99, 99), 6 * 512 * PE_CYC,
                                lambda s=s: emit_out_unit(s)])
            state["fill_total"] += 4 * 6 * 512 * PE_CYC

    for p in range(NPAIR):
        qp = qkT[2 * p]
        kp = qkT[2 * p + 1]
        for t in range(N_QT):
            maybe_fill(force_deadline=(p, t))
            n_kc = 2 * t + 2
            av2 = ps_av.tile([65, 2, QT], F32, tag="av", name=f"av{p}_{t}")
            pt_tiles = [None] * n_kc

            def emit_scores(k):
                r = k - 2 * t
                off = 0 if r < 0 else KC * r
                w = QT - off
                # padded so the two heads' slices sit in separate PSUM banks
                # (tile_position matmuls sharing a bank crash walrus codegen)
                sc = ps_sc.tile([KC, 2, QT], F32, tag="sc", name=f"sc{p}_{t}_{k}",
                                padded_shape=[KC, 2, 2 * QT])
                pt2 = pT_p.tile([KC, 2, QT], BF16, tag="pT", name=f"pt{p}_{t}_{k}")
                for j in (0, 1):
                    nc.tensor.matmul(
                        sc[:, j, 0:w],
                        kp[j * 64:(j + 1) * 64, k * KC:(k + 1) * KC],
                        qp[j * 64:(j + 1) * 64, t * QT + off:(t + 1) * QT],
                        start=True, stop=True, tile_position=(j * 64, 0))
                nc.scalar.activation(pt2[:, :, 0:w], sc[:, :, 0:w],
                                     mybir.ActivationFunctionType.Exp)
                if r >= 0:
                    msk2 = bass.AP(tensor=mask_t.tensor, offset=mask_t.offset,
                                   ap=[list(mask_t.ap[0]), [0, 2],
                                       list(mask_t.ap[1])])
                    nc.vector.tensor_mul(pt2[:, :, 0:KC], pt2[:, :, 0:KC], msk2)
                pt_tiles[k] = (pt2, off, w)

            def emit_av(k):
                # av2 is one 2KB PSUM zero region: open the accumulation
                # group once (k==0, j==0) and close it once (last, j==1)
                pt2, off, w = pt_tiles[k]
                for j in (0, 1):
                    nc.tensor.matmul(
                        av2[:, j, off:off + w], vsb[k][:, 2 * p + j, :],
                        pt2[:, j, 0:w],
                        start=(k == 0 and j == 0),
                        stop=(k == n_kc - 1 and j == 1))
                pt_tiles[k] = None

            for k in range(n_kc):
                emit_scores(k)
                r = k - 2 * t
                w = QT if r < 0 else QT - KC * r
                state["act_done"] += 2 * w * ACT_CYC + 143.0
                if k == 1 and pending_norm[0] is not None:
                    emit_norm_b(*pending_norm[0])
                    pending_norm[0] = None
                if k >= 1:
                    emit_av(k - 1)
                maybe_fill()
            emit_av(n_kc - 1)
            rc = emit_norm_a(av2)
            pending_norm[0] = (p, t, av2, rc)

    emit_norm_b(*pending_norm[0])
    pending_norm[0] = None
    maybe_fill(force_deadline=(99, 99))
    assert not fillers


def make_pools(tc, ctx):
    consts = ctx.enter_context(tc.tile_pool(name="consts", bufs=1))
    wqk_p = ctx.enter_context(tc.tile_pool(name="wqk", bufs=6))
    wv_p = ctx.enter_context(tc.tile_pool(name="wv", bufs=6))
    wo_p = ctx.enter_context(tc.tile_pool(name="wo", bufs=3))
    xt_p = ctx.enter_context(tc.tile_pool(name="xt", bufs=6))
    qkT_p = ctx.enter_context(tc.tile_pool(name="qkT", bufs=6))
    vsb_p = ctx.enter_context(tc.tile_pool(name="vsb", bufs=16))
    pT_p = ctx.enter_context(tc.tile_pool(name="pT", bufs=3))
    apair_p = ctx.enter_context(tc.tile_pool(name="apair", bufs=3))
    rc_p = ctx.enter_context(tc.tile_pool(name="rc", bufs=2))
    bcsb_p = ctx.enter_context(tc.tile_pool(name="bcsb", bufs=2))
    at_p = ctx.enter_context(tc.tile_pool(name="at", bufs=2))
    outsb_p = ctx.enter_context(tc.tile_pool(name="outsb", bufs=2))
    ps_sc = ctx.enter_context(tc.tile_pool(name="ps_sc", bufs=2, space="PSUM"))
    ps_av = ctx.enter_context(tc.tile_pool(name="ps_av", bufs=2, space="PSUM"))
    ps_ms = ctx.enter_context(tc.tile_pool(name="ps_ms", bufs=2, space="PSUM"))
    return (consts, wqk_p, wv_p, wo_p, xt_p, qkT_p, vsb_p, pT_p, apair_p,
            rc_p, bcsb_p, at_p, outsb_p, ps_sc, ps_av, ps_ms)


def build_nc(n_iters=None, phases=None):
    from contextlib import ExitStack

    nc = bacc.Bacc(trn_type="TRN2", debug=False)
    nc._allow_low_precision_reason = "bf16 kept within 2e-2 tolerance"
    io = declare_io(nc)
    with tile.TileContext(nc) as tc:
        with ExitStack() as ctx:
            pools = make_pools(tc, ctx)
            if n_iters is None:
                build_body(nc, tc, pools, io)
            else:
                with tc.For_i(0, n_iters, 1):
                    build_body(nc, tc, pools, io)
    nc.compile()
    return nc, io


def host_inputs(x, W_qkv, b_qkv, W_out, b_out):
    """Per-core in_maps + the host-side unshard constant."""
    bf16 = mybir.dt.np(BF16)
    x = np.asarray(x, dtype=np.float32)
    W_qkv = np.asarray(W_qkv, dtype=np.float32)
    b_qkv = np.asarray(b_qkv, dtype=np.float32)
    W_out = np.asarray(W_out, dtype=np.float32)
    b_out = np.asarray(b_out, dtype=np.float32)

    Wq, Wk, Wv = W_qkv[:, 0:D], W_qkv[:, D:2 * D], W_qkv[:, 2 * D:3 * D]
    bq, bk, bv = b_qkv[0:D], b_qkv[D:2 * D], b_qkv[2 * D:3 * D]
    scale = 1.0 / np.sqrt(DH)

    mask = (np.arange(KC)[None, :] >= np.arange(KC)[:, None]).astype(bf16)
    sel = np.zeros((65, 64), bf16)
    sel[64, :] = 1.0
    onesc = np.ones((128, HPG), bf16)

    per_group = []
    for g in range(G):
        cols, bcols = [], []
        for p in range(NPAIR):
            h0 = g * HPG + 2 * p
            cols.append(Wq[:, h0 * DH:(h0 + 2) * DH] * scale)
            cols.append(Wk[:, h0 * DH:(h0 + 2) * DH])
            bcols.append(bq[h0 * DH:(h0 + 2) * DH] * scale)
            bcols.append(bk[h0 * DH:(h0 + 2) * DH])
        wqk_g = np.concatenate(cols, axis=1).astype(bf16)       # [768, 768]
        bqk_g = np.stack(bcols, axis=1).astype(np.float32)      # [128, 6]
        wv_g = np.zeros((D, WVC), np.float32)
        for h in range(HPG):
            hg = g * HPG + h
            wv_g[:, h * 65:h * 65 + DH] = Wv[:, hg * DH:(hg + 1) * DH]
        wo_g = W_out[g * HPG * DH:(g + 1) * HPG * DH, :].astype(bf16)
        per_group.append((wqk_g, bqk_g, wv_g.astype(bf16), wo_g))

    xt_b = [np.ascontiguousarray(x[b].T).astype(bf16) for b in range(B)]

    in_maps = []
    for core in range(N_CORES):
        b, g = core // G, core % G
        wqk_g, bqk_g, wv_g, wo_g = per_group[g]
        in_maps.append(dict(
            xt=xt_b[b], wqk=wqk_g, bqk2=bqk_g, wv=wv_g, wo=wo_g,
            mask=mask, sel=sel, onesc=onesc,
        ))
    cvec = (bv @ W_out + b_out).astype(np.float32)              # [768]
    return in_maps, cvec


_CACHE = {}


def kernel(x, W_qkv, b_qkv, W_out, b_out):
    from concourse.bass_utils import run_bass_kernel_spmd

    if "nc" not in _CACHE:
        _CACHE["nc"], _ = build_nc()
    nc = _CACHE["nc"]
    in_maps, cvec = host_inputs(x, W_qkv, b_qkv, W_out, b_out)
    res = run_bass_kernel_spmd(nc, in_maps, list(range(N_CORES)))
    out = np.empty((B, S, D), np.float32)
    for b in range(B):
        out[b] = res.results[2 * b]["out"] + res.results[2 * b + 1]["out"] + cvec
    return out
